# revision 1
# baseline (speedup 1.0000x reference)
"""MoE top-2 routing kernel for TRN2, 8 NeuronCores, expert-parallel.

Strategy (one expert per core, E == n_cores == 8):
  - Router: each core computes fp32 gate scores for its 1/8 token shard
    (host supplies the shard pre-transposed [F, B/8]), builds the dense
    per-token gate row [tok, 8] via the DVE max8 instruction, and the 8
    cores AllGather the full [B, 8] gate matrix.
  - Compaction: every core extracts its own expert's gate column, builds
    the compacted token list for that expert with a matmul-based
    exclusive cumsum over the 0/1 mask, and indirect-DMA scatters
    (token_id, segment_row, gate) triples into a fixed-capacity list.
  - Expert MLP: gather token rows from a replicated bf16 copy of the
    features, PE-transpose to feature-major, then two bf16 matmuls
    (fp32 PSUM accumulate) with ReLU between, bias adds, and gate scale.
  - Combine: scatter gate-scaled rows into a zeroed per-segment partial
    [B/S, O]; ReduceScatter(add) per segment sums the two expert
    contributions per token across cores. Host stitches the shards.

The token axis is split into S=2 segments so the first segment's
ReduceScatter overlaps the second segment's compute.
"""

import os
import sys

for _p in ("/opt/trn_rl_repo",):
    if _p not in sys.path and os.path.isdir(_p):
        sys.path.insert(0, _p)

import numpy as np
import ml_dtypes

import concourse.bass as bass
import concourse.mybir as mybir
import concourse.tile as tile
from concourse import bacc
from concourse.bass import IndirectOffsetOnAxis
from concourse.bass_utils import run_bass_kernel_spmd

FP32 = mybir.dt.float32
BF16 = mybir.dt.bfloat16
I32 = mybir.dt.int32
AF = mybir.ActivationFunctionType
ALU = mybir.AluOpType


# ---------------------------------------------------------------------------
# Configuration (full problem size; sim tests pass a scaled-down cfg)
# ---------------------------------------------------------------------------
def full_cfg():
    return dict(
        n_cores=8,
        E=8,
        B=8192,
        F=2048,
        H=8192,
        O=2048,
        S=2,           # token segments (RS per segment for comm/compute overlap)
        C_seg=1152,    # per-(segment, expert) token slot capacity (mult of TB)
        TB=384,        # token block for the MLP matmuls (moving dim)
        h_super=512,   # H columns per W1 slab (4 PSUM tiles of 128)
        o_super=512,   # O columns per W2 slab
        w2_hgrp=16,    # H chunks per W2 slab group
    )


def _strip_intra_group_deps(group):
    """Remove WAW sync edges between instructions in `group` (their writes
    target disjoint rows of one tensor, so pairwise ordering is unneeded and
    each serialized hop costs a DMA-completion semaphore round-trip)."""
    names = {bi.ins.name for bi in group}
    for bi in group:
        for dep in list(bi.ins.sync_dependency_names()):
            if dep in names:
                bi.ins.try_remove_dependency(dep)


def build_nc(cfg):
    n_cores = cfg["n_cores"]
    E = cfg["E"]
    B, F, H, O = cfg["B"], cfg["F"], cfg["H"], cfg["O"]
    S, C_seg, TB = cfg["S"], cfg["C_seg"], cfg["TB"]
    h_super, o_super, w2_hgrp = cfg["h_super"], cfg["o_super"], cfg["w2_hgrp"]

    Fc, Hc, Oc = F // 128, H // 128, O // 128
    Btok = B // n_cores          # tokens per core shard (router)
    Bseg = B // S                # tokens per segment
    ncols = B // 128             # mask columns (token t = col*128 + p)
    cols_seg = ncols // S
    Rseg = Bseg // n_cores       # RS output rows per core per segment
    n_blocks = C_seg // TB
    assert C_seg % TB == 0 and TB % 128 == 0 and TB <= 512
    assert H % h_super == 0 and O % o_super == 0 and Hc % w2_hgrp == 0
    gtb = TB // 128              # 128-sub-tiles per token block

    rg = [list(range(n_cores))]

    nc = bacc.Bacc(
        "TRN2", debug=False, target_bir_lowering=False, num_devices=n_cores
    )

    # ---------------- external inputs ----------------
    xT_shard = nc.dram_tensor("xT_shard", [F, Btok], FP32, kind="ExternalInput")
    feat_bf16 = nc.dram_tensor("feat_bf16", [B, F], BF16, kind="ExternalInput")
    gate_W = nc.dram_tensor("gate_W", [F, E], FP32, kind="ExternalInput")
    gate_b = nc.dram_tensor("gate_b", [E], FP32, kind="ExternalInput")
    expert_bias = nc.dram_tensor("expert_bias", [E], FP32, kind="ExternalInput")
    W1 = nc.dram_tensor("W1", [F, H], BF16, kind="ExternalInput")
    W2 = nc.dram_tensor("W2", [H, O], BF16, kind="ExternalInput")
    b1T = nc.dram_tensor("b1T", [128, Hc], FP32, kind="ExternalInput")
    b2T = nc.dram_tensor("b2T", [128, Oc], FP32, kind="ExternalInput")
    onehot = nc.dram_tensor("onehot", [128, E], FP32, kind="ExternalInput")
    ident128_bf16 = nc.dram_tensor("ident128_bf16", [128, 128], BF16,
                                   kind="ExternalInput")
    ident128_f32 = nc.dram_tensor("ident128_f32", [128, 128], FP32,
                                  kind="ExternalInput")
    identE_f32 = nc.dram_tensor("identE_f32", [E, E], FP32, kind="ExternalInput")
    identC_f32 = nc.dram_tensor("identC_f32", [ncols, ncols], FP32,
                                kind="ExternalInput")
    ustrict128 = nc.dram_tensor("ustrict128", [128, 128], FP32,
                                kind="ExternalInput")
    useg = nc.dram_tensor("useg", [ncols, ncols], FP32, kind="ExternalInput")
    ones_col = nc.dram_tensor("ones_col", [128, 1], FP32, kind="ExternalInput")
    ones_row = nc.dram_tensor("ones_row", [1, 128], FP32, kind="ExternalInput")
    iota_p = nc.dram_tensor("iota_p", [128, 1], FP32, kind="ExternalInput")
    iota_rows = nc.dram_tensor("iota_rows", [128, 128], FP32,
                               kind="ExternalInput")

    # ---------------- external output ----------------
    out_ext = nc.dram_tensor("out", [S * Rseg, O], BF16, kind="ExternalOutput")
    debug = cfg.get("debug", False)
    if debug:
        dbg_gate = nc.dram_tensor("dbg_gate", [B, E], FP32,
                                  kind="ExternalOutput")
        dbg_parts = [nc.dram_tensor(f"dbg_partial_{s}", [Bseg + 128, O], BF16,
                                    kind="ExternalOutput") for s in range(S)]
        dbg_lgs = [nc.dram_tensor(f"dbg_lg_{s}", [C_seg, 3], FP32,
                                  kind="ExternalOutput") for s in range(S)]

    # ---------------- internal DRAM ----------------
    gate_local = nc.dram_tensor("gate_local", [Btok, E], FP32)
    gate_all = nc.dram_tensor("gate_all", [B, E], FP32, addr_space="Shared")
    # +128 pad rows: empty slots land there (no bounds reg needed)
    partials = [nc.dram_tensor(f"partial_{s}", [Bseg + 128, O], BF16)
                for s in range(S)]
    rs_outs = [nc.dram_tensor(f"rs_out_{s}", [Rseg, O], BF16)
               for s in range(S)]

    with tile.TileContext(nc) as tc:
        # ------------- constants into SBUF -------------
        cpool = tc.alloc_tile_pool(name="consts", bufs=1)
        idb = cpool.tile([128, 128], BF16, name="idb")
        idf = cpool.tile([128, 128], FP32, name="idf")
        ide = cpool.tile([E, E], FP32, name="ide")
        idc = cpool.tile([ncols, ncols], FP32, name="idc")
        ustr = cpool.tile([128, 128], FP32, name="ustr")
        usg = cpool.tile([ncols, ncols], FP32, name="usg")
        onec = cpool.tile([128, 1], FP32, name="onec")
        oner = cpool.tile([1, 128], FP32, name="oner")
        iop = cpool.tile([128, 1], FP32, name="iop")
        ior = cpool.tile([128, 128], FP32, name="ior")
        ohr = cpool.tile([128, E], FP32, name="ohr")
        b1s = cpool.tile([128, Hc], FP32, name="b1s")
        b2s = cpool.tile([128, Oc], FP32, name="b2s")
        nc.sync.dma_start(out=idb[:], in_=ident128_bf16[:, :])
        nc.sync.dma_start(out=idf[:], in_=ident128_f32[:, :])
        nc.sync.dma_start(out=ide[:], in_=identE_f32[:, :])
        nc.sync.dma_start(out=idc[:], in_=identC_f32[:, :])
        nc.sync.dma_start(out=ustr[:], in_=ustrict128[:, :])
        nc.sync.dma_start(out=usg[:], in_=useg[:, :])
        nc.sync.dma_start(out=onec[:], in_=ones_col[:, :])
        nc.sync.dma_start(out=oner[:], in_=ones_row[:, :])
        nc.sync.dma_start(out=iop[:], in_=iota_p[:, :])
        nc.sync.dma_start(out=ior[:], in_=iota_rows[:, :])
        nc.sync.dma_start(out=ohr[:], in_=onehot[:, :])
        nc.sync.dma_start(out=b1s[:], in_=b1T[:, :])
        nc.sync.dma_start(out=b2s[:], in_=b2T[:, :])


        # ------------- phase A: router on the local token shard -------------
        with tc.tile_pool(name="rsb", bufs=1) as rsb, \
             tc.tile_pool(name="rps", bufs=2, space="PSUM") as rps:
            gw = rsb.tile([128, Fc, E], FP32, name="gw")
            nc.sync.dma_start(
                out=gw[:], in_=gate_W[:, :].rearrange("(c p) e -> p c e", p=128)
            )
            gb = rsb.tile([E, 1], FP32, name="gb")
            eb = rsb.tile([E, 1], FP32, name="eb")
            nc.sync.dma_start(out=gb[:], in_=gate_b[:, None])
            nc.sync.dma_start(out=eb[:], in_=expert_bias[:, None])
            cb = rsb.tile([E, 1], FP32, name="cb")
            nc.vector.tensor_add(out=cb[:], in0=gb[:], in1=eb[:])

            xts = rsb.tile([128, Fc, Btok], FP32, name="xts")
            nc.sync.dma_start(
                out=xts[:], in_=xT_shard[:, :].rearrange("(c p) t -> p c t", p=128)
            )

            sT = rsb.tile([E, Btok], FP32, name="sT")
            nbw = min(512, Btok)
            for nb in range(Btok // nbw):
                ps = rps.tile([E, nbw], FP32, name="ps_sc")
                for f in range(Fc):
                    nc.tensor.matmul(
                        out=ps[:],
                        lhsT=gw[:, f, :],
                        rhs=xts[:, f, nb * nbw:(nb + 1) * nbw],
                        start=(f == 0),
                        stop=(f == Fc - 1),
                    )
                nc.scalar.activation(
                    out=sT[:, nb * nbw:(nb + 1) * nbw], in_=ps[:],
                    func=AF.Identity, bias=cb[:],
                )

            # per 128-token tile: transpose scores, top-2 gate row
            gl = rsb.tile([128, Btok // 128, E], FP32, name="gl")
            for i in range(Btok // 128):
                pst = rps.tile([128, E], FP32, name="ps_tr")
                nc.tensor.transpose(
                    out=pst[:], in_=sT[:, i * 128:(i + 1) * 128], identity=ide[:]
                )
                sc = rsb.tile([128, E], FP32, name="sc")
                nc.vector.tensor_copy(out=sc[:], in_=pst[:])
                s8 = rsb.tile([128, 8], FP32, name="s8")
                nc.vector.max(out=s8[:], in_=sc[:])
                d = rsb.tile([128, 1], FP32, name="d")
                nc.vector.tensor_sub(out=d[:], in0=s8[:, 0:1], in1=s8[:, 1:2])
                w1t = rsb.tile([128, 1], FP32, name="w1t")
                w2t = rsb.tile([128, 1], FP32, name="w2t")
                nc.scalar.activation(out=w1t[:], in_=d[:], func=AF.Sigmoid)
                nc.scalar.activation(out=w2t[:], in_=d[:], func=AF.Sigmoid,
                                     scale=-1.0)
                eq1 = rsb.tile([128, E], FP32, name="eq1")
                eq2 = rsb.tile([128, E], FP32, name="eq2")
                nc.vector.tensor_tensor(
                    out=eq1[:], in0=sc[:], in1=s8[:, 0:1].to_broadcast([128, E]),
                    op=ALU.is_equal,
                )
                nc.vector.tensor_tensor(
                    out=eq2[:], in0=sc[:], in1=s8[:, 1:2].to_broadcast([128, E]),
                    op=ALU.is_equal,
                )
                g1 = rsb.tile([128, E], FP32, name="g1")
                nc.vector.tensor_scalar_mul(g1[:], eq1[:], w1t[:])
                nc.vector.scalar_tensor_tensor(
                    out=gl[:, i, :], in0=eq2[:], scalar=w2t[:], in1=g1[:],
                    op0=ALU.mult, op1=ALU.add,
                )
            nc.sync.dma_start(
                out=gate_local[:, :].rearrange("(n p) e -> p n e", p=128),
                in_=gl[:],
            )

            nc.gpsimd.collective_compute(
                "AllGather",
                ALU.bypass,
                replica_groups=rg,
                ins=[gate_local[:, :].opt()],
                outs=[gate_all[:, :].opt()],
            )
            if debug:
                for r in range(B // 128):
                    t3 = rsb.tile([128, E], FP32, name="dt3")
                    nc.sync.dma_start(
                        out=t3[:], in_=gate_all[r * 128:(r + 1) * 128, :])
                    nc.sync.dma_start(
                        out=dbg_gate[r * 128:(r + 1) * 128, :], in_=t3[:])

        # ------------- phase B: compaction for this core's expert -------------
        lists_sb = {}
        clpool = tc.alloc_tile_pool(name="clpool", bufs=1)
        # token t = c*128 + p ; segment s(c) = c // cols_seg
        with tc.tile_pool(name="csb", bufs=1) as csb, \
             tc.tile_pool(name="cps", bufs=1, space="PSUM") as cps:
            ga = csb.tile([128, ncols, E], FP32, name="ga")
            nc.sync.dma_start(
                out=ga[:], in_=gate_all[:, :].rearrange("(n p) e -> p n e", p=128)
            )
            gcol = csb.tile([128, ncols], FP32, name="gcol")
            for c in range(ncols):
                t8 = csb.tile([128, E], FP32, name="t8")
                nc.vector.tensor_tensor(
                    out=t8[:], in0=ga[:, c, :], in1=ohr[:], op=ALU.mult
                )
                nc.vector.reduce_sum(
                    out=gcol[:, c:c + 1], in_=t8[:], axis=mybir.AxisListType.X
                )
            mask = csb.tile([128, ncols], FP32, name="mask")
            nc.vector.tensor_scalar(
                out=mask[:], in0=gcol[:], scalar1=0.0, scalar2=None, op0=ALU.is_gt
            )
            # exclusive cumsum across partitions within each column
            pw = cps.tile([128, ncols], FP32, name="pw")
            nc.tensor.matmul(out=pw[:], lhsT=ustr[:], rhs=mask[:],
                             start=True, stop=True)
            # column totals -> exclusive cumsum across columns (seg-blocked)
            ptot = cps.tile([1, ncols], FP32, name="ptot")
            nc.tensor.matmul(out=ptot[:], lhsT=onec[:], rhs=mask[:],
                             start=True, stop=True)
            tot_sb = csb.tile([1, ncols], FP32, name="tot_sb")
            nc.vector.tensor_copy(out=tot_sb[:], in_=ptot[:])
            pcol = cps.tile([ncols, 1], FP32, name="pcol")
            nc.tensor.matmul(out=pcol[:], lhsT=tot_sb[:], rhs=oner[:, 0:1],
                             start=True, stop=True)
            tcol_sb = csb.tile([ncols, 1], FP32, name="tcol_sb")
            nc.vector.tensor_copy(out=tcol_sb[:], in_=pcol[:])
            pex = cps.tile([ncols, 1], FP32, name="pex")
            nc.tensor.matmul(out=pex[:], lhsT=usg[:], rhs=tcol_sb[:],
                             start=True, stop=True)
            ex_sb = csb.tile([ncols, 1], FP32, name="ex_sb")
            nc.vector.tensor_copy(out=ex_sb[:], in_=pex[:])
            pexr = cps.tile([1, ncols], FP32, name="pexr")
            nc.tensor.matmul(out=pexr[:], lhsT=ex_sb[:], rhs=idc[:],
                             start=True, stop=True)
            exr_sb = csb.tile([1, ncols], FP32, name="exr_sb")
            nc.vector.tensor_copy(out=exr_sb[:], in_=pexr[:])
            pbc = cps.tile([128, ncols], FP32, name="pbc")
            nc.tensor.matmul(out=pbc[:], lhsT=oner[:], rhs=exr_sb[:],
                             start=True, stop=True)
            pw_sb = csb.tile([128, ncols], FP32, name="pw_sb")
            nc.vector.tensor_copy(out=pw_sb[:], in_=pw[:])
            pos = csb.tile([128, ncols], FP32, name="pos")
            nc.vector.tensor_add(out=pos[:], in0=pw_sb[:], in1=pbc[:])
            # unrouted tokens -> pad slot C_seg: off = mask*(pos-C) + C
            off = csb.tile([128, ncols], FP32, name="off")
            nc.vector.tensor_scalar_add(off[:], pos[:], float(-C_seg))
            nc.vector.tensor_tensor(out=off[:], in0=off[:], in1=mask[:],
                                    op=ALU.mult)
            nc.vector.tensor_scalar_add(off[:], off[:], float(C_seg))


            # vals per column: (global token id, segment row, gate)
            vals_all = csb.tile([128, ncols, 3], FP32, name="vals_all")
            for c in range(ncols):
                s = c // cols_seg
                nc.vector.tensor_scalar_add(vals_all[:, c, 0:1], iop[:],
                                            float(c * 128))
                nc.vector.tensor_scalar_add(vals_all[:, c, 1:2], iop[:],
                                            float(c * 128 - s * Bseg))
                nc.vector.tensor_copy(vals_all[:, c, 2:3], gcol[:, c:c + 1])
            # build list tiles in SBUF: one-hot select via is_equal + matmul
            eqpool = tc.alloc_tile_pool(name="eqpool", bufs=8)
            for s in range(S):
                for g in range(C_seg // 128):
                    iog = csb.tile([128, 128], FP32, name="iog")
                    nc.vector.tensor_scalar_add(iog[:], ior[:], float(g * 128))
                    pl = cps.tile([128, 3], FP32, name="pl")
                    for j, c in enumerate(range(s * cols_seg,
                                                (s + 1) * cols_seg)):
                        eq = eqpool.tile([128, 128], FP32, name="eq")
                        nc.vector.tensor_tensor(
                            out=eq[:],
                            in0=off[:, c:c + 1].to_broadcast([128, 128]),
                            in1=iog[:], op=ALU.is_equal,
                        )
                        nc.tensor.matmul(
                            out=pl[:], lhsT=eq[:], rhs=vals_all[:, c, :],
                            start=(j == 0), stop=(j == cols_seg - 1),
                        )
                    lsb = clpool.tile([128, 3], FP32, name=f"lsb_{s}_{g}",
                                      tag=f"lsb_{s}_{g}")
                    nc.vector.tensor_copy(out=lsb[:], in_=pl[:])
                    # empty slots (gate==0): send scatter row to pad row Bseg
                    eq0 = csb.tile([128, 1], FP32, name="eq0")
                    nc.vector.tensor_scalar(
                        out=eq0[:], in0=lsb[:, 2:3], scalar1=0.0, scalar2=None,
                        op0=ALU.is_equal,
                    )
                    nc.vector.scalar_tensor_tensor(
                        out=lsb[:, 1:2], in0=eq0[:], scalar=float(Bseg),
                        in1=lsb[:, 1:2], op0=ALU.mult, op1=ALU.add,
                    )
                    gi = clpool.tile([128, 1], I32, name=f"gi_{s}_{g}",
                                     tag=f"gi_{s}_{g}")
                    si = clpool.tile([128, 1], I32, name=f"si_{s}_{g}",
                                     tag=f"si_{s}_{g}")
                    nc.vector.tensor_copy(out=gi[:], in_=lsb[:, 0:1])
                    nc.vector.tensor_copy(out=si[:], in_=lsb[:, 1:2])
                    lists_sb[(s, g)] = (gi, si, lsb)

            eqpool.release()

        # ------------- zero the partials (overlaps phase C prologue) ---------
        with tc.tile_pool(name="z2pool", bufs=1) as zp2:
            zt = zp2.tile([128, O], BF16, name="zt")
            nc.vector.memset(zt[:], 0.0)
            _zero_grp = []
            for s in range(S):
                for r in range((Bseg + 128) // 128):
                    _zero_grp.append(nc.sync.dma_start(
                        out=partials[s][r * 128:(r + 1) * 128, :], in_=zt[:]
                    ))
            _strip_intra_group_deps(_zero_grp)

        # ------------- phase C: expert MLP per segment -------------
        mm_sb = tc.alloc_tile_pool(name="mm_sb", bufs=1)
        xpool = tc.alloc_tile_pool(name="xpool", bufs=2)
        wpool = tc.alloc_tile_pool(name="wpool", bufs=2)
        gpool = tc.alloc_tile_pool(name="gpool", bufs=2)
        ypool = tc.alloc_tile_pool(name="ypool", bufs=2)
        pmm1 = tc.alloc_tile_pool(name="pmm1", bufs=2, space="PSUM")
        pmm2 = tc.alloc_tile_pool(name="pmm2", bufs=1, space="PSUM")
        ptr = tc.alloc_tile_pool(name="ptr", bufs=2, space="PSUM")

        W1r = W1[:, :].rearrange("(c p) h -> p c h", p=128)
        W2r = W2[:, :].rearrange("(c p) o -> p c o", p=128)

        for s in range(S):
            _ysc_grp = []
            for b in range(n_blocks):
                base = b * TB
                xT = xpool.tile([128, Fc, TB], BF16, name="xT")
                grow = gpool.tile([1, TB], FP32, name="grow")
                sidx_blk = []
                for g in range(gtb):
                    gidx, sidx, lsb = lists_sb[(s, b * gtb + g)]
                    sidx_blk.append(sidx)
                    xraw = gpool.tile([128, F], BF16, name="xraw")
                    nc.gpsimd.indirect_dma_start(
                        out=xraw[:],
                        out_offset=None,
                        in_=feat_bf16[:, :],
                        in_offset=IndirectOffsetOnAxis(ap=gidx[:], axis=0),
                    )
                    for f in range(Fc):
                        pt = ptr.tile([128, 128], BF16, name="pt_x", tag="pt")
                        nc.tensor.transpose(
                            out=pt[:], in_=xraw[:, f * 128:(f + 1) * 128],
                            identity=idb[:],
                        )
                        nc.vector.tensor_copy(
                            out=xT[:, f, g * 128:(g + 1) * 128], in_=pt[:]
                        )
                    pgr = ptr.tile([1, 128], FP32, name="pgr", tag="pt")
                    nc.tensor.matmul(out=pgr[:], lhsT=lsb[:, 2:3], rhs=idf[:],
                                     start=True, stop=True)
                    nc.vector.tensor_copy(
                        out=grow[:, g * 128:(g + 1) * 128], in_=pgr[:]
                    )
                pgrep = ptr.tile([128, TB], FP32, name="pgrep", tag="pt")
                nc.tensor.matmul(out=pgrep[:], lhsT=oner[:], rhs=grow[:],
                                 start=True, stop=True)
                grep = gpool.tile([128, TB], FP32, name="grep")
                nc.vector.tensor_copy(out=grep[:], in_=pgrep[:])

                # ---- MM1: hT = relu(x @ W1 + b1), feature-major ----
                hT = mm_sb.tile([128, Hc, TB], BF16, name="hT")
                for hs in range(H // h_super):
                    w1s = wpool.tile([128, Fc, h_super], BF16, name="w1s")
                    nc.sync.dma_start(
                        out=w1s[:],
                        in_=W1r[:, :, hs * h_super:(hs + 1) * h_super],
                    )
                    for ht in range(h_super // 128):
                        hg_i = hs * (h_super // 128) + ht
                        p1 = pmm1.tile([128, TB], FP32, name="p1")
                        for f in range(Fc):
                            nc.tensor.matmul(
                                out=p1[:],
                                lhsT=w1s[:, f, ht * 128:(ht + 1) * 128],
                                rhs=xT[:, f, :],
                                start=(f == 0),
                                stop=(f == Fc - 1),
                            )
                        nc.scalar.activation(
                            out=hT[:, hg_i, :], in_=p1[:], func=AF.Relu,
                            bias=b1s[:, hg_i:hg_i + 1],
                        )

                # ---- MM2: y = h @ W2 + b2, then gate scale + transpose ----
                youts = [ypool.tile([128, Oc * 128], BF16, name=f"yout{g}",
                                    tag=f"yout{g}") for g in range(gtb)]
                n_hgrp = Hc // w2_hgrp
                for os_ in range(O // o_super):
                    p2s = [pmm2.tile([128, TB], FP32, name=f"p2_{ot}")
                           for ot in range(o_super // 128)]
                    for hg in range(n_hgrp):
                        w2s = wpool.tile([128, w2_hgrp, o_super], BF16,
                                         name="w2s")
                        nc.sync.dma_start(
                            out=w2s[:],
                            in_=W2r[:, hg * w2_hgrp:(hg + 1) * w2_hgrp,
                                    os_ * o_super:(os_ + 1) * o_super],
                        )
                        for ot in range(o_super // 128):
                            for hh in range(w2_hgrp):
                                nc.tensor.matmul(
                                    out=p2s[ot][:],
                                    lhsT=w2s[:, hh, ot * 128:(ot + 1) * 128],
                                    rhs=hT[:, hg * w2_hgrp + hh, :],
                                    start=(hg == 0 and hh == 0),
                                    stop=(hg == n_hgrp - 1 and hh == w2_hgrp - 1),
                                )
                    for ot in range(o_super // 128):
                        o_i = os_ * (o_super // 128) + ot
                        yb = ypool.tile([128, TB], FP32, name="yb")
                        nc.scalar.activation(
                            out=yb[:], in_=p2s[ot][:], func=AF.Identity,
                            bias=b2s[:, o_i:o_i + 1],
                        )
                        ysc = ypool.tile([128, TB], BF16, name="ysc")
                        nc.vector.tensor_mul(out=ysc[:], in0=yb[:], in1=grep[:])
                        for g in range(gtb):
                            pt2 = ptr.tile([128, 128], BF16, name="pt_y", tag="pt")
                            nc.tensor.transpose(
                                out=pt2[:], in_=ysc[:, g * 128:(g + 1) * 128],
                                identity=idb[:],
                            )
                            nc.vector.tensor_copy(
                                out=youts[g][:, o_i * 128:(o_i + 1) * 128],
                                in_=pt2[:]
                            )
                for g in range(gtb):
                    _ysc_grp.append(nc.gpsimd.indirect_dma_start(
                        out=partials[s][:, :],
                        out_offset=IndirectOffsetOnAxis(ap=sidx_blk[g][:],
                                                        axis=0),
                        in_=youts[g][:],
                        in_offset=None,
                    ))

            if debug:
                with tc.tile_pool(name=f"dbgp_{s}", bufs=2) as dp:
                    for r in range((Bseg + 128) // 128):
                        t = dp.tile([128, O], BF16, name="dt")
                        nc.sync.dma_start(
                            out=t[:], in_=partials[s][r * 128:(r + 1) * 128, :])
                        nc.sync.dma_start(
                            out=dbg_parts[s][r * 128:(r + 1) * 128, :], in_=t[:])
                    for g in range(C_seg // 128):
                        nc.sync.dma_start(
                            out=dbg_lgs[s][g * 128:(g + 1) * 128, :],
                            in_=lists_sb[(s, g)][2][:])
            _strip_intra_group_deps(_ysc_grp)
            nc.gpsimd.collective_compute(
                "ReduceScatter",
                ALU.add,
                replica_groups=rg,
                ins=[partials[s][0:Bseg, :].opt()],
                outs=[rs_outs[s][:, :].opt()],
            )
            nc.sync.dma_start(
                out=out_ext[s * Rseg:(s + 1) * Rseg, :], in_=rs_outs[s][:, :]
            )

        for _pool in (ptr, pmm2, pmm1, ypool, gpool, wpool, xpool, mm_sb,
                      clpool, cpool):
            _pool.release()

    nc.compile()
    return nc


# ---------------------------------------------------------------------------
# Host side
# ---------------------------------------------------------------------------
def make_in_maps(cfg, features, gate_W, gate_b, expert_bias, W1, b1, W2, b2):
    n_cores = cfg["n_cores"]
    B, F, H, O, E = cfg["B"], cfg["F"], cfg["H"], cfg["O"], cfg["E"]
    S, ncols = cfg["S"], cfg["B"] // 128
    cols_seg = ncols // S
    Btok = B // n_cores
    Fc, Hc, Oc = F // 128, H // 128, O // 128
    bf16 = ml_dtypes.bfloat16

    feat_bf16 = np.ascontiguousarray(features.astype(bf16))
    ident128 = np.eye(128, dtype=np.float32)
    consts = dict(
        gate_W=np.ascontiguousarray(gate_W.astype(np.float32)),
        gate_b=np.ascontiguousarray(gate_b.astype(np.float32)),
        expert_bias=np.ascontiguousarray(expert_bias.astype(np.float32)),
        feat_bf16=feat_bf16,
        ident128_bf16=np.ascontiguousarray(ident128.astype(bf16)),
        ident128_f32=ident128,
        identE_f32=np.eye(E, dtype=np.float32),
        identC_f32=np.eye(ncols, dtype=np.float32),
        ustrict128=np.triu(np.ones((128, 128), np.float32), 1),
        useg=np.triu(np.ones((ncols, ncols), np.float32), 1)
        * (np.arange(ncols)[:, None] // cols_seg
           == np.arange(ncols)[None, :] // cols_seg).astype(np.float32),
        ones_col=np.ones((128, 1), np.float32),
        ones_row=np.ones((1, 128), np.float32),
        iota_p=np.arange(128, dtype=np.float32).reshape(128, 1),
        iota_rows=np.tile(np.arange(128, dtype=np.float32), (128, 1)),
    )
    in_maps = []
    for c in range(n_cores):
        m = dict(consts)
        m["xT_shard"] = np.ascontiguousarray(
            features[c * Btok:(c + 1) * Btok, :].T.astype(np.float32)
        )
        m["W1"] = np.ascontiguousarray(W1[c].astype(bf16))
        m["W2"] = np.ascontiguousarray(W2[c].astype(bf16))
        m["b1T"] = np.ascontiguousarray(
            b1[c].astype(np.float32).reshape(Hc, 128).T
        )
        m["b2T"] = np.ascontiguousarray(
            b2[c].astype(np.float32).reshape(Oc, 128).T
        )
        oh = np.zeros((128, E), np.float32)
        oh[:, c] = 1.0
        m["onehot"] = oh
        in_maps.append(m)
    return in_maps


def assemble_output(cfg, results):
    n_cores, B, O, S = cfg["n_cores"], cfg["B"], cfg["O"], cfg["S"]
    Bseg = B // S
    Rseg = Bseg // n_cores
    out = np.empty((B, O), np.float32)
    for c in range(n_cores):
        o = np.asarray(results[c]["out"]).astype(np.float32)
        for s in range(S):
            out[s * Bseg + c * Rseg: s * Bseg + (c + 1) * Rseg, :] = \
                o[s * Rseg:(s + 1) * Rseg, :]
    return out


_NC_CACHE = {}


def _get_nc(cfg_key_cfg):
    key = tuple(sorted(cfg_key_cfg.items()))
    if key not in _NC_CACHE:
        _NC_CACHE[key] = build_nc(cfg_key_cfg)
    return _NC_CACHE[key]


def run(inputs, trace=False, cfg=None):
    cfg = cfg or full_cfg()
    nc = _get_nc(cfg)
    in_maps = make_in_maps(
        cfg,
        np.asarray(inputs["features"]), np.asarray(inputs["gate_W"]),
        np.asarray(inputs["gate_b"]), np.asarray(inputs["expert_bias"]),
        np.asarray(inputs["W1"]), np.asarray(inputs["b1"]),
        np.asarray(inputs["W2"]), np.asarray(inputs["b2"]),
    )
    res = run_bass_kernel_spmd(
        nc, in_maps, core_ids=list(range(cfg["n_cores"])), trace=trace
    )
    out = assemble_output(cfg, res.results)
    return out, res


def kernel(**inputs):
    out, _ = run(inputs, trace=False)
    return out

